# revision 26
# baseline (speedup 1.0000x reference)
"""LoRA QKV parallel linear with per-token slot routing, on 8 TRN2 NeuronCores.

Data-parallel over tokens (8192 -> 1024 per core), weights replicated.
All matmul operands are bf16 (tolerance 2e-2 >> bf16 matmul error ~5e-3),
which halves HBM traffic and SBUF footprint vs f32 and enables the fast
weight-load path. PSUM accumulation stays fp32.

Per core:
  phase 1: hT[(g,l,r), t] = A^T-chunk-stationary matmuls over k, directly in
           the [rank, token] orientation (no PE transposes), then one
           elementwise multiply with a host-precomputed routing mask
           m[(l,r), t] = scale[l] * (slot[t] == l)  (scale folded into B
           host-side, so m is one-hot {0,1}).
  phase 2: for each 128-token tile i: out[t, o] accumulates
           sum_k x[k,i]^T @ W[k, o] over 16 k-chunks into 6 PSUM banks
           (one per 512-wide o-chunk), closed by the LoRA delta matmul
           hT-slice-stationary @ B. PSUM -> SBUF copies alternate between
           the Scalar and Vector engines; output is written bf16 and
           upcast on host.
"""

import numpy as np
import ml_dtypes

import concourse.bass as bass
import concourse.bacc as bacc
import concourse.mybir as mybir
import concourse.tile as tile

HIDDEN = 2048
Q_SIZE = 2048
KV_SIZE = 512
OUT = Q_SIZE + 2 * KV_SIZE  # 3072
MAX_LORAS = 8
RANK = 16
T = 8192
N_CORES = 8
T_CORE = T // N_CORES  # 1024

P = 128
KC = HIDDEN // P          # 16 k-chunks
OJ = OUT // 512           # 6 output chunks of 512
NT = T_CORE // P          # 8 token tiles per core
GR = MAX_LORAS * RANK     # 128 = all (slot, rank) pairs for one target group
F32 = mybir.dt.float32
BF16 = mybir.dt.bfloat16
NPBF16 = ml_dtypes.bfloat16

_NC_CACHE = {}


def build_nc():
    """Build the SPMD Bass program (same program on every core)."""
    nc = bacc.Bacc("TRN2", target_bir_lowering=False, debug=False, num_devices=N_CORES)

    # x/A/w are pre-swizzled host-side so every DMA moves multi-KB contiguous
    # per-partition lines (descriptor efficiency sets input bandwidth):
    #   xQ[m, p, c*1024+t] = x^T[(2m+c)*128+p, t]            (4 KB lines)
    #   aQ[m, p, c*384+gr] = A^T[(2m+c)*128+p, gr]           (1.5 KB lines)
    #   wJ[j, p, k*512+o]  = W^T[k*128+p, j*512+o]           (8 KB lines)
    xQ = nc.dram_tensor("xQ", [KC // 2, P, 2 * T_CORE], BF16,
                        kind="ExternalInput").ap()
    aQ = nc.dram_tensor("aQ", [KC // 2, P, 2 * 3 * GR], BF16,
                        kind="ExternalInput").ap()
    wJ = nc.dram_tensor("wJ", [OJ, P, KC * 512], BF16, kind="ExternalInput").ap()
    bqkv = nc.dram_tensor("bqkv", [GR, OUT], BF16, kind="ExternalInput").ap()
    mask = nc.dram_tensor("mask", [GR, T_CORE], BF16, kind="ExternalInput").ap()
    yb = nc.dram_tensor("yb", [T_CORE, OUT], BF16, kind="ExternalOutput").ap()

    with tile.TileContext(nc) as tc:
        with (
            tc.tile_pool(name="xsb", bufs=1) as xpool,
            tc.tile_pool(name="wsb", bufs=1) as wpool,
            tc.tile_pool(name="asb", bufs=1) as apool,
            tc.tile_pool(name="bsb", bufs=1) as bpool,
            tc.tile_pool(name="msb", bufs=1) as mpool,
            tc.tile_pool(name="ht", bufs=1) as htpool,
            tc.tile_pool(name="o", bufs=12) as opool,
            tc.tile_pool(name="ps", bufs=1, space="PSUM") as pspool,
        ):
            # --- resident inputs (DMA'd in chunks; tile deps order the MMs) ---
            # Issue order matters: phase 1 streams x/A chunks immediately;
            # mask+B are needed at the first delta (~+35us); w chunk j is
            # needed only when phase 2 reaches o-chunk j (j-outer loop).
            xsb = xpool.tile([P, KC * T_CORE], BF16)    # free idx = k*1024 + t
            asb = apool.tile([P, KC * 3 * GR], BF16)    # free idx = k*384 + g*128 + lr
            msb = mpool.tile([P, T_CORE], BF16)         # routing mask [lr, t]
            bsb = bpool.tile([P, OUT], BF16)            # [lr, o] = bq|bk|bv
            # w stored per o-chunk: free idx = j*(16*512) + k*512 + o.
            # 4 k-chunks per DMA: 4 KB contiguous per partition line.
            wsb = wpool.tile([P, KC * OUT], BF16)

            def dma_w(j, kq):   # kq in 0..1, 8 k-chunks each
                nc.sync.dma_start(
                    wsb[:, (j * KC + kq * 8) * 512:(j * KC + (kq + 1) * 8) * 512],
                    wJ[j, :, kq * 8 * 512:(kq + 1) * 8 * 512])

            def dma_xa(m):      # k-chunk pair m covers k = 2m, 2m+1
                nc.sync.dma_start(
                    xsb[:, 2 * m * T_CORE:2 * (m + 1) * T_CORE], xQ[m, :, :])
                nc.sync.dma_start(
                    asb[:, 2 * m * 3 * GR:2 * (m + 1) * 3 * GR], aQ[m, :, :])

            # Issue order = arrival order. Phase 1 consumes x/A chunk k at
            # ~1.3us per chunk; w_j0 must be complete when phase 2 starts
            # (right at phase-1 end), so its sub-DMAs go mid-stream. The
            # first pair is split in half so phase 1 starts sooner.
            nc.sync.dma_start(xsb[:, 0:T_CORE], xQ[0, :, 0:T_CORE])
            nc.sync.dma_start(asb[:, 0:3 * GR], aQ[0, :, 0:3 * GR])
            nc.sync.dma_start(xsb[:, T_CORE:2 * T_CORE], xQ[0, :, T_CORE:])
            nc.sync.dma_start(asb[:, 3 * GR:2 * 3 * GR], aQ[0, :, 3 * GR:])
            for m in range(1, 4):
                dma_xa(m)
            nc.sync.dma_start(msb[:], mask[:, :])
            nc.sync.dma_start(bsb[:], bqkv[:, :])
            dma_w(0, 0)
            for m in range(4, 6):
                dma_xa(m)
            dma_w(0, 1)
            for m in range(6, 8):
                dma_xa(m)
            for j in range(1, OJ):
                for kq in range(2):
                    dma_w(j, kq)

            # --- PE warm-up: ~16 junk matmuls on a zeroed tile while the
            # first input DMAs land, so the HAM clock-gate is at full rate
            # when the real stream starts ---
            warm = mpool.tile([P, 512], BF16, name="warm_src")
            nc.vector.memset(warm[:], 0.0)
            wps = pspool.tile([P, 512], F32, name="warm_ps", tag="ps6")
            for _ in range(16):
                nc.tensor.matmul(wps[:], lhsT=warm[:, 0:P], rhs=warm[:],
                                 start=True, stop=True)

            # --- phase 1: hT[g][lr, t], A-chunk stationary, x moving;
            # each (g,th) bank is masked (one-hot) + downcast to bf16 on DVE
            # as soon as its accumulation closes ---
            hps = [pspool.tile([P, 512], F32, name=f"hps{n}", tag=f"ps{n}") for n in range(6)]
            hT = htpool.tile([P, 3 * T_CORE], BF16)     # free idx = g*1024 + t
            for k in range(KC):
                for g in range(3):
                    for th in range(2):
                        nc.tensor.matmul(
                            hps[g * 2 + th][:],
                            lhsT=asb[:, k * 3 * GR + g * GR:
                                     k * 3 * GR + (g + 1) * GR],
                            rhs=xsb[:, k * T_CORE + th * 512:
                                    k * T_CORE + (th + 1) * 512],
                            start=(k == 0), stop=(k == KC - 1))
                        if k == KC - 1:
                            nc.vector.tensor_tensor(
                                hT[:, g * T_CORE + th * 512:
                                   g * T_CORE + (th + 1) * 512],
                                hps[g * 2 + th][:],
                                msb[:, th * 512:(th + 1) * 512],
                                op=mybir.AluOpType.mult)

            # --- phase 2: y tile [t128, o512]; j-outer so w chunk j is only
            # needed once phase 2 reaches it (relaxes the DMA deadline) ---
            # delta source group per o-chunk j: q,q,q,q,k,v
            jg = [0, 0, 0, 0, 1, 2]
            for j in range(OJ):
                for i in range(NT):
                    n = j * NT + i
                    ops = pspool.tile([P, 512], F32, name=f"ops{n}",
                                      tag=f"ps{(n + 6) % 8}")
                    for k in range(KC):
                        nc.tensor.matmul(
                            ops[:],
                            lhsT=xsb[:, k * T_CORE + i * P:
                                     k * T_CORE + (i + 1) * P],
                            rhs=wsb[:, (j * KC + k) * 512:
                                    (j * KC + k + 1) * 512],
                            start=(k == 0), stop=False)
                    nc.tensor.matmul(
                        ops[:],
                        lhsT=hT[:, jg[j] * T_CORE + i * P:
                                jg[j] * T_CORE + (i + 1) * P],
                        rhs=bsb[:, j * 512:(j + 1) * 512],
                        start=False, stop=True)
                    osb = opool.tile([P, 512], BF16, name=f"osb{i}_{j}", tag="o")
                    nc.vector.tensor_copy(osb[:], ops[:])
                    nc.sync.dma_start(
                        yb[i * P:(i + 1) * P, j * 512:(j + 1) * 512], osb[:])
    nc.compile()
    return nc


def prep_in_maps(x, weight, lora_A, lora_B_q, lora_B_k, lora_B_v,
                 lora_scaling, token_to_slot):
    x = np.asarray(x, dtype=np.float32)
    weight = np.asarray(weight, dtype=np.float32)
    lora_A = np.asarray(lora_A, dtype=np.float32)
    lora_B_q = np.asarray(lora_B_q, dtype=np.float32)
    lora_B_k = np.asarray(lora_B_k, dtype=np.float32)
    lora_B_v = np.asarray(lora_B_v, dtype=np.float32)
    lora_scaling = np.asarray(lora_scaling, dtype=np.float32)
    slot = np.asarray(token_to_slot).astype(np.int64)

    xT = np.ascontiguousarray(x.astype(NPBF16).T)       # (2048, 8192) bf16
    # wJ[j, p, k*512+o] = W^T[k*128+p, j*512+o]: 16 KB contiguous per
    # (j, partition) so the per-j DMA uses full-size descriptors.
    wT = weight.astype(NPBF16).T                        # (2048, 3072) bf16
    wJ = np.ascontiguousarray(
        wT.reshape(KC, P, OJ, 512).transpose(2, 1, 0, 3).reshape(OJ, P, KC * 512))
    # aT col = g*128 + l*16 + r; aQ pairs k-chunks for 1.5 KB lines
    aT = lora_A.transpose(1, 0, 2, 3).reshape(3 * GR, HIDDEN).T.astype(NPBF16)
    aQ = np.ascontiguousarray(
        aT.reshape(KC // 2, 2, P, 3 * GR).transpose(0, 2, 1, 3)
        .reshape(KC // 2, P, 2 * 3 * GR))
    # b row = l*16 + r, scaling folded in; columns = q | k | v
    bq = (lora_scaling[:, None, None] * lora_B_q).transpose(0, 2, 1).reshape(GR, Q_SIZE)
    bk = (lora_scaling[:, None, None] * lora_B_k).transpose(0, 2, 1).reshape(GR, KV_SIZE)
    bv = (lora_scaling[:, None, None] * lora_B_v).transpose(0, 2, 1).reshape(GR, KV_SIZE)
    bqkv = np.ascontiguousarray(
        np.concatenate([bq, bk, bv], axis=1).astype(NPBF16))  # (128, 3072)
    # routing mask [l*16+r, t]: 1 where slot[t] == l (scale already in B)
    onehot = (slot[None, :] == np.arange(MAX_LORAS)[:, None])          # (8, T)
    mask = np.repeat(onehot, RANK, axis=0).astype(NPBF16)              # (128, T)

    in_maps = []
    for c in range(N_CORES):
        sl = slice(c * T_CORE, (c + 1) * T_CORE)
        xc = xT[:, sl]  # (2048, 1024)
        xQ = np.ascontiguousarray(
            xc.reshape(KC // 2, 2, P, T_CORE).transpose(0, 2, 1, 3)
            .reshape(KC // 2, P, 2 * T_CORE))
        in_maps.append({
            "xQ": xQ,
            "wJ": wJ,
            "aQ": aQ,
            "bqkv": bqkv,
            "mask": np.ascontiguousarray(mask[:, sl]),
        })
    return in_maps


def kernel(**inputs):
    from concourse.bass_utils import run_bass_kernel_spmd
    if "nc" not in _NC_CACHE:
        _NC_CACHE["nc"] = build_nc()
    nc = _NC_CACHE["nc"]
    in_maps = prep_in_maps(**inputs)
    res = run_bass_kernel_spmd(nc, in_maps, core_ids=list(range(N_CORES)))
    return np.concatenate(
        [r["yb"].astype(np.float32) for r in res.results], axis=0)


# revision 27
# speedup vs baseline: 1.0018x; 1.0018x over previous
"""LoRA QKV parallel linear with per-token slot routing, on 8 TRN2 NeuronCores.

Data-parallel over tokens (8192 -> 1024 per core), weights replicated.
All matmul operands are bf16 (tolerance 2e-2 >> bf16 matmul error ~5e-3),
which halves HBM traffic and SBUF footprint vs f32 and enables the fast
weight-load path. PSUM accumulation stays fp32.

Per core:
  phase 1: hT[(g,l,r), t] = A^T-chunk-stationary matmuls over k, directly in
           the [rank, token] orientation (no PE transposes), then one
           elementwise multiply with a host-precomputed routing mask
           m[(l,r), t] = scale[l] * (slot[t] == l)  (scale folded into B
           host-side, so m is one-hot {0,1}).
  phase 2: for each 128-token tile i: out[t, o] accumulates
           sum_k x[k,i]^T @ W[k, o] over 16 k-chunks into 6 PSUM banks
           (one per 512-wide o-chunk), closed by the LoRA delta matmul
           hT-slice-stationary @ B. PSUM -> SBUF copies alternate between
           the Scalar and Vector engines; output is written bf16 and
           upcast on host.
"""

import numpy as np
import ml_dtypes

import concourse.bass as bass
import concourse.bacc as bacc
import concourse.mybir as mybir
import concourse.tile as tile

HIDDEN = 2048
Q_SIZE = 2048
KV_SIZE = 512
OUT = Q_SIZE + 2 * KV_SIZE  # 3072
MAX_LORAS = 8
RANK = 16
T = 8192
N_CORES = 8
T_CORE = T // N_CORES  # 1024

P = 128
KC = HIDDEN // P          # 16 k-chunks
OJ = OUT // 512           # 6 output chunks of 512
NT = T_CORE // P          # 8 token tiles per core
GR = MAX_LORAS * RANK     # 128 = all (slot, rank) pairs for one target group
F32 = mybir.dt.float32
BF16 = mybir.dt.bfloat16
NPBF16 = ml_dtypes.bfloat16

_NC_CACHE = {}


def build_nc():
    """Build the SPMD Bass program (same program on every core)."""
    nc = bacc.Bacc("TRN2", target_bir_lowering=False, debug=False, num_devices=N_CORES)

    # x/A/w are pre-swizzled host-side so every DMA moves multi-KB contiguous
    # per-partition lines (descriptor efficiency sets input bandwidth):
    #   xQ[m, p, c*1024+t] = x^T[(2m+c)*128+p, t]            (4 KB lines)
    #   aQ[m, p, c*384+gr] = A^T[(2m+c)*128+p, gr]           (1.5 KB lines)
    #   wJ[j, p, k*512+o]  = W^T[k*128+p, j*512+o]           (8 KB lines)
    xQ = nc.dram_tensor("xQ", [KC // 2, P, 2 * T_CORE], BF16,
                        kind="ExternalInput").ap()
    aQ = nc.dram_tensor("aQ", [KC // 2, P, 2 * 3 * GR], BF16,
                        kind="ExternalInput").ap()
    wJ = nc.dram_tensor("wJ", [OJ, P, KC * 512], BF16, kind="ExternalInput").ap()
    bqkv = nc.dram_tensor("bqkv", [GR, OUT], BF16, kind="ExternalInput").ap()
    mask = nc.dram_tensor("mask", [GR, T_CORE], BF16, kind="ExternalInput").ap()
    yb = nc.dram_tensor("yb", [T_CORE, OUT], BF16, kind="ExternalOutput").ap()

    with tile.TileContext(nc) as tc:
        with (
            tc.tile_pool(name="xsb", bufs=1) as xpool,
            tc.tile_pool(name="wsb", bufs=1) as wpool,
            tc.tile_pool(name="asb", bufs=1) as apool,
            tc.tile_pool(name="bsb", bufs=1) as bpool,
            tc.tile_pool(name="msb", bufs=1) as mpool,
            tc.tile_pool(name="ht", bufs=1) as htpool,
            tc.tile_pool(name="o", bufs=12) as opool,
            tc.tile_pool(name="ps", bufs=1, space="PSUM") as pspool,
        ):
            # --- resident inputs (DMA'd in chunks; tile deps order the MMs) ---
            # Issue order matters: phase 1 streams x/A chunks immediately;
            # mask+B are needed at the first delta (~+35us); w chunk j is
            # needed only when phase 2 reaches o-chunk j (j-outer loop).
            xsb = xpool.tile([P, KC * T_CORE], BF16)    # free idx = k*1024 + t
            asb = apool.tile([P, KC * 3 * GR], BF16)    # free idx = k*384 + g*128 + lr
            msb = mpool.tile([P, T_CORE], BF16)         # routing mask [lr, t]
            bsb = bpool.tile([P, OUT], BF16)            # [lr, o] = bq|bk|bv
            # w stored per o-chunk: free idx = j*(16*512) + k*512 + o.
            # 4 k-chunks per DMA: 4 KB contiguous per partition line.
            wsb = wpool.tile([P, KC * OUT], BF16)

            def dma_w(j, kq):   # kq in 0..1, 8 k-chunks each
                nc.sync.dma_start(
                    wsb[:, (j * KC + kq * 8) * 512:(j * KC + (kq + 1) * 8) * 512],
                    wJ[j, :, kq * 8 * 512:(kq + 1) * 8 * 512])

            def dma_xa(m):      # k-chunk pair m covers k = 2m, 2m+1
                nc.sync.dma_start(
                    xsb[:, 2 * m * T_CORE:2 * (m + 1) * T_CORE], xQ[m, :, :])
                nc.sync.dma_start(
                    asb[:, 2 * m * 3 * GR:2 * (m + 1) * 3 * GR], aQ[m, :, :])

            # Issue order = arrival order. Phase 1 consumes x/A chunk k at
            # ~1.3us per chunk; w_j0 must be complete when phase 2 starts
            # (right at phase-1 end), so its sub-DMAs go mid-stream. The
            # first pair is split in half so phase 1 starts sooner.
            nc.sync.dma_start(xsb[:, 0:T_CORE], xQ[0, :, 0:T_CORE])
            nc.sync.dma_start(asb[:, 0:3 * GR], aQ[0, :, 0:3 * GR])
            nc.sync.dma_start(xsb[:, T_CORE:2 * T_CORE], xQ[0, :, T_CORE:])
            nc.sync.dma_start(asb[:, 3 * GR:2 * 3 * GR], aQ[0, :, 3 * GR:])
            for m in range(1, 4):
                dma_xa(m)
            nc.sync.dma_start(msb[:], mask[:, :])
            nc.sync.dma_start(bsb[:], bqkv[:, :])
            dma_w(0, 0)
            for m in range(4, 6):
                dma_xa(m)
            dma_w(0, 1)
            for m in range(6, 8):
                dma_xa(m)
            for j in range(1, OJ):
                for kq in range(2):
                    dma_w(j, kq)

            # --- PE warm-up: ~16 junk matmuls on a zeroed tile while the
            # first input DMAs land, so the HAM clock-gate is at full rate
            # when the real stream starts ---
            warm = mpool.tile([P, 512], BF16, name="warm_src")
            nc.vector.memset(warm[:], 0.0)
            wps = pspool.tile([P, 512], F32, name="warm_ps", tag="ps6")
            for _ in range(16):
                nc.tensor.matmul(wps[:], lhsT=warm[:, 0:P], rhs=warm[:],
                                 start=True, stop=True)

            # --- phase 1: hT[g][lr, t], A-chunk stationary, x moving;
            # each (g,th) bank is masked (one-hot) + downcast to bf16 on DVE
            # as soon as its accumulation closes ---
            hps = [pspool.tile([P, 512], F32, name=f"hps{n}", tag=f"ps{n}") for n in range(6)]
            hT = htpool.tile([P, 3 * T_CORE], BF16)     # free idx = g*1024 + t
            for k in range(KC):
                for g in range(3):
                    for th in range(2):
                        nc.tensor.matmul(
                            hps[g * 2 + th][:],
                            lhsT=asb[:, k * 3 * GR + g * GR:
                                     k * 3 * GR + (g + 1) * GR],
                            rhs=xsb[:, k * T_CORE + th * 512:
                                    k * T_CORE + (th + 1) * 512],
                            start=(k == 0), stop=(k == KC - 1))
                        if k == KC - 1:
                            nc.vector.tensor_tensor(
                                hT[:, g * T_CORE + th * 512:
                                   g * T_CORE + (th + 1) * 512],
                                hps[g * 2 + th][:],
                                msb[:, th * 512:(th + 1) * 512],
                                op=mybir.AluOpType.mult)

            # --- phase 2: y tile [t128, o512]; j-outer so w chunk j is only
            # needed once phase 2 reaches it (relaxes the DMA deadline) ---
            # delta source group per o-chunk j: q,q,q,q,k,v
            jg = [0, 0, 0, 0, 1, 2]
            for j in range(OJ):
                for i in range(NT):
                    n = j * NT + i
                    ops = pspool.tile([P, 512], F32, name=f"ops{n}",
                                      tag=f"ps{(n + 6) % 8}")
                    for k in range(KC):
                        nc.tensor.matmul(
                            ops[:],
                            lhsT=xsb[:, k * T_CORE + i * P:
                                     k * T_CORE + (i + 1) * P],
                            rhs=wsb[:, (j * KC + k) * 512:
                                    (j * KC + k + 1) * 512],
                            start=(k == 0), stop=False)
                    nc.tensor.matmul(
                        ops[:],
                        lhsT=hT[:, jg[j] * T_CORE + i * P:
                                jg[j] * T_CORE + (i + 1) * P],
                        rhs=bsb[:, j * 512:(j + 1) * 512],
                        start=False, stop=True)
                    osb = opool.tile([P, 512], BF16, name=f"osb{i}_{j}", tag="o")
                    if n % 2 == 0:
                        nc.scalar.copy(osb[:], ops[:])
                    else:
                        nc.vector.tensor_copy(osb[:], ops[:])
                    nc.sync.dma_start(
                        yb[i * P:(i + 1) * P, j * 512:(j + 1) * 512], osb[:])
    nc.compile()
    return nc


def prep_in_maps(x, weight, lora_A, lora_B_q, lora_B_k, lora_B_v,
                 lora_scaling, token_to_slot):
    x = np.asarray(x, dtype=np.float32)
    weight = np.asarray(weight, dtype=np.float32)
    lora_A = np.asarray(lora_A, dtype=np.float32)
    lora_B_q = np.asarray(lora_B_q, dtype=np.float32)
    lora_B_k = np.asarray(lora_B_k, dtype=np.float32)
    lora_B_v = np.asarray(lora_B_v, dtype=np.float32)
    lora_scaling = np.asarray(lora_scaling, dtype=np.float32)
    slot = np.asarray(token_to_slot).astype(np.int64)

    xT = np.ascontiguousarray(x.astype(NPBF16).T)       # (2048, 8192) bf16
    # wJ[j, p, k*512+o] = W^T[k*128+p, j*512+o]: 16 KB contiguous per
    # (j, partition) so the per-j DMA uses full-size descriptors.
    wT = weight.astype(NPBF16).T                        # (2048, 3072) bf16
    wJ = np.ascontiguousarray(
        wT.reshape(KC, P, OJ, 512).transpose(2, 1, 0, 3).reshape(OJ, P, KC * 512))
    # aT col = g*128 + l*16 + r; aQ pairs k-chunks for 1.5 KB lines
    aT = lora_A.transpose(1, 0, 2, 3).reshape(3 * GR, HIDDEN).T.astype(NPBF16)
    aQ = np.ascontiguousarray(
        aT.reshape(KC // 2, 2, P, 3 * GR).transpose(0, 2, 1, 3)
        .reshape(KC // 2, P, 2 * 3 * GR))
    # b row = l*16 + r, scaling folded in; columns = q | k | v
    bq = (lora_scaling[:, None, None] * lora_B_q).transpose(0, 2, 1).reshape(GR, Q_SIZE)
    bk = (lora_scaling[:, None, None] * lora_B_k).transpose(0, 2, 1).reshape(GR, KV_SIZE)
    bv = (lora_scaling[:, None, None] * lora_B_v).transpose(0, 2, 1).reshape(GR, KV_SIZE)
    bqkv = np.ascontiguousarray(
        np.concatenate([bq, bk, bv], axis=1).astype(NPBF16))  # (128, 3072)
    # routing mask [l*16+r, t]: 1 where slot[t] == l (scale already in B)
    onehot = (slot[None, :] == np.arange(MAX_LORAS)[:, None])          # (8, T)
    mask = np.repeat(onehot, RANK, axis=0).astype(NPBF16)              # (128, T)

    in_maps = []
    for c in range(N_CORES):
        sl = slice(c * T_CORE, (c + 1) * T_CORE)
        xc = xT[:, sl]  # (2048, 1024)
        xQ = np.ascontiguousarray(
            xc.reshape(KC // 2, 2, P, T_CORE).transpose(0, 2, 1, 3)
            .reshape(KC // 2, P, 2 * T_CORE))
        in_maps.append({
            "xQ": xQ,
            "wJ": wJ,
            "aQ": aQ,
            "bqkv": bqkv,
            "mask": np.ascontiguousarray(mask[:, sl]),
        })
    return in_maps


def kernel(**inputs):
    from concourse.bass_utils import run_bass_kernel_spmd
    if "nc" not in _NC_CACHE:
        _NC_CACHE["nc"] = build_nc()
    nc = _NC_CACHE["nc"]
    in_maps = prep_in_maps(**inputs)
    res = run_bass_kernel_spmd(nc, in_maps, core_ids=list(range(N_CORES)))
    return np.concatenate(
        [r["yb"].astype(np.float32) for r in res.results], axis=0)


# revision 29
# speedup vs baseline: 1.0198x; 1.0180x over previous
"""LoRA QKV parallel linear with per-token slot routing, on 8 TRN2 NeuronCores.

Data-parallel over tokens (8192 -> 1024 per core), weights replicated.
All matmul operands are bf16 (tolerance 2e-2 >> bf16 matmul error ~3.5e-3),
which halves HBM traffic / SBUF footprint vs f32 and enables the fast
weight-load path; PSUM accumulation stays fp32. The kernel is PE-bound:
912 matmuls x 512 cols ~= 197us/core at full rate, and the schedule keeps
the PE array >90% busy.

Per core:
  warm-up: ~16 junk matmuls on a zeroed tile while the first input DMAs
           land, so the HAM clock-gate is at full rate for the real stream.
  phase 1: hT[(g,l,r), t] = A-chunk-stationary matmuls over k, directly in
           the [rank, token] orientation (no PE transposes); each bank is
           masked with a host-precomputed one-hot routing mask
           m[(l,r), t] = (slot[t] == l) and downcast to bf16 on DVE as soon
           as its accumulation closes (lora_scaling is folded into B).
  phase 2: j-outer over 512-wide o-chunks so w chunk j is only needed when
           phase 2 reaches it; per 128-token tile i, 16 k-chunk matmuls
           accumulate into one PSUM bank (8-bank rotation, offset so the
           first blocks don't wait on the mask ops), closed by the LoRA
           delta matmul (hT-slice stationary @ B). Output is written bf16
           and upcast on host.

Inputs are host-swizzled so every DMA moves multi-KB contiguous
per-partition lines (descriptor efficiency sets the effective input
bandwidth), and the issue order matches the consumption order.
"""

import numpy as np
import ml_dtypes

import concourse.bacc as bacc
import concourse.mybir as mybir
import concourse.tile as tile

HIDDEN = 2048
Q_SIZE = 2048
KV_SIZE = 512
OUT = Q_SIZE + 2 * KV_SIZE  # 3072
MAX_LORAS = 8
RANK = 16
T = 8192
N_CORES = 8
T_CORE = T // N_CORES  # 1024

P = 128
KC = HIDDEN // P          # 16 k-chunks
OJ = OUT // 512           # 6 output chunks of 512
NT = T_CORE // P          # 8 token tiles per core
GR = MAX_LORAS * RANK     # 128 = all (slot, rank) pairs for one target group
F32 = mybir.dt.float32
BF16 = mybir.dt.bfloat16
NPBF16 = ml_dtypes.bfloat16

_NC_CACHE = {}


def build_nc():
    """Build the SPMD Bass program (same program on every core)."""
    nc = bacc.Bacc("TRN2", target_bir_lowering=False, debug=False, num_devices=N_CORES)

    # x/A/w are pre-swizzled host-side so every DMA moves multi-KB contiguous
    # per-partition lines (descriptor efficiency sets input bandwidth):
    #   xQ[m, p, c*1024+t] = x^T[(2m+c)*128+p, t]            (4 KB lines)
    #   aQ[m, p, c*384+gr] = A^T[(2m+c)*128+p, gr]           (1.5 KB lines)
    #   wJ[j, p, k*512+o]  = W^T[k*128+p, j*512+o]           (8 KB lines)
    xQ = nc.dram_tensor("xQ", [KC // 2, P, 2 * T_CORE], BF16,
                        kind="ExternalInput").ap()
    aQ = nc.dram_tensor("aQ", [KC // 2, P, 2 * 3 * GR], BF16,
                        kind="ExternalInput").ap()
    wJ = nc.dram_tensor("wJ", [OJ, P, KC * 512], BF16, kind="ExternalInput").ap()
    bqkv = nc.dram_tensor("bqkv", [GR, OUT], BF16, kind="ExternalInput").ap()
    mask = nc.dram_tensor("mask", [GR, T_CORE], BF16, kind="ExternalInput").ap()
    yb = nc.dram_tensor("yb", [T_CORE, OUT], BF16, kind="ExternalOutput").ap()

    with tile.TileContext(nc) as tc:
        with (
            tc.tile_pool(name="xsb", bufs=1) as xpool,
            tc.tile_pool(name="wsb", bufs=1) as wpool,
            tc.tile_pool(name="asb", bufs=1) as apool,
            tc.tile_pool(name="bsb", bufs=1) as bpool,
            tc.tile_pool(name="msb", bufs=1) as mpool,
            tc.tile_pool(name="ht", bufs=1) as htpool,
            tc.tile_pool(name="o", bufs=12) as opool,
            tc.tile_pool(name="ps", bufs=1, space="PSUM") as pspool,
        ):
            # --- resident inputs (DMA'd in chunks; tile deps order the MMs) ---
            # Issue order matters: phase 1 streams x/A chunks immediately;
            # mask+B are needed at the first delta (~+35us); w chunk j is
            # needed only when phase 2 reaches o-chunk j (j-outer loop).
            xsb = xpool.tile([P, KC * T_CORE], BF16)    # free idx = k*1024 + t
            asb = apool.tile([P, KC * 3 * GR], BF16)    # free idx = k*384 + g*128 + lr
            msb = mpool.tile([P, T_CORE], BF16)         # routing mask [lr, t]
            bsb = bpool.tile([P, OUT], BF16)            # [lr, o] = bq|bk|bv
            # w stored per o-chunk: free idx = j*(16*512) + k*512 + o.
            # 4 k-chunks per DMA: 4 KB contiguous per partition line.
            wsb = wpool.tile([P, KC * OUT], BF16)

            def dma_w(j, kq):   # kq in 0..1, 8 k-chunks each
                nc.sync.dma_start(
                    wsb[:, (j * KC + kq * 8) * 512:(j * KC + (kq + 1) * 8) * 512],
                    wJ[j, :, kq * 8 * 512:(kq + 1) * 8 * 512])

            def dma_xa(m):      # k-chunk pair m covers k = 2m, 2m+1
                nc.sync.dma_start(
                    xsb[:, 2 * m * T_CORE:2 * (m + 1) * T_CORE], xQ[m, :, :])
                nc.sync.dma_start(
                    asb[:, 2 * m * 3 * GR:2 * (m + 1) * 3 * GR], aQ[m, :, :])

            # Issue order = arrival order. Phase 1 consumes x/A chunk k at
            # ~1.3us per chunk; w_j0 must be complete when phase 2 starts
            # (right at phase-1 end), so its sub-DMAs go mid-stream. The
            # first pair is split in half so phase 1 starts sooner.
            nc.sync.dma_start(xsb[:, 0:T_CORE], xQ[0, :, 0:T_CORE])
            nc.sync.dma_start(asb[:, 0:3 * GR], aQ[0, :, 0:3 * GR])
            nc.sync.dma_start(xsb[:, T_CORE:2 * T_CORE], xQ[0, :, T_CORE:])
            nc.sync.dma_start(asb[:, 3 * GR:2 * 3 * GR], aQ[0, :, 3 * GR:])
            for m in range(1, 4):
                dma_xa(m)
            nc.sync.dma_start(msb[:], mask[:, :])
            nc.sync.dma_start(bsb[:], bqkv[:, :])
            dma_w(0, 0)
            for m in range(4, 6):
                dma_xa(m)
            dma_w(0, 1)
            for m in range(6, 8):
                dma_xa(m)
            for j in range(1, OJ):
                for kq in range(2):
                    dma_w(j, kq)

            # --- PE warm-up: ~16 junk matmuls on a zeroed tile while the
            # first input DMAs land, so the HAM clock-gate is at full rate
            # when the real stream starts ---
            warm = mpool.tile([P, 512], BF16, name="warm_src")
            nc.vector.memset(warm[:], 0.0)
            wps = pspool.tile([P, 512], F32, name="warm_ps", tag="ps6")
            for _ in range(16):
                nc.tensor.matmul(wps[:], lhsT=warm[:, 0:P], rhs=warm[:],
                                 start=True, stop=True)

            # --- phase 1: hT[g][lr, t], A-chunk stationary, x moving;
            # each (g,th) bank is masked (one-hot) + downcast to bf16 on DVE
            # as soon as its accumulation closes ---
            hps = [pspool.tile([P, 512], F32, name=f"hps{n}", tag=f"ps{n}") for n in range(6)]
            hT = htpool.tile([P, 3 * T_CORE], BF16)     # free idx = g*1024 + t
            for k in range(KC):
                for g in range(3):
                    for th in range(2):
                        nc.tensor.matmul(
                            hps[g * 2 + th][:],
                            lhsT=asb[:, k * 3 * GR + g * GR:
                                     k * 3 * GR + (g + 1) * GR],
                            rhs=xsb[:, k * T_CORE + th * 512:
                                    k * T_CORE + (th + 1) * 512],
                            start=(k == 0), stop=(k == KC - 1))
                        if k == KC - 1:
                            nc.vector.tensor_tensor(
                                hT[:, g * T_CORE + th * 512:
                                   g * T_CORE + (th + 1) * 512],
                                hps[g * 2 + th][:],
                                msb[:, th * 512:(th + 1) * 512],
                                op=mybir.AluOpType.mult)

            # --- phase 2: y tile [t128, o512]; j-outer so w chunk j is only
            # needed once phase 2 reaches it (relaxes the DMA deadline) ---
            # delta source group per o-chunk j: q,q,q,q,k,v
            jg = [0, 0, 0, 0, 1, 2]
            for j in range(OJ):
                for i in range(NT):
                    n = j * NT + i
                    ops = pspool.tile([P, 512], F32, name=f"ops{n}",
                                      tag=f"ps{(n + 6) % 8}")
                    for k in range(KC):
                        nc.tensor.matmul(
                            ops[:],
                            lhsT=xsb[:, k * T_CORE + i * P:
                                     k * T_CORE + (i + 1) * P],
                            rhs=wsb[:, (j * KC + k) * 512:
                                    (j * KC + k + 1) * 512],
                            start=(k == 0), stop=False)
                    nc.tensor.matmul(
                        ops[:],
                        lhsT=hT[:, jg[j] * T_CORE + i * P:
                                jg[j] * T_CORE + (i + 1) * P],
                        rhs=bsb[:, j * 512:(j + 1) * 512],
                        start=False, stop=True)
                    osb = opool.tile([P, 512], BF16, name=f"osb{i}_{j}", tag="o")
                    nc.vector.tensor_copy(osb[:], ops[:])
                    nc.sync.dma_start(
                        yb[i * P:(i + 1) * P, j * 512:(j + 1) * 512], osb[:])
    nc.compile()
    return nc


def prep_in_maps(x, weight, lora_A, lora_B_q, lora_B_k, lora_B_v,
                 lora_scaling, token_to_slot):
    x = np.asarray(x, dtype=np.float32)
    weight = np.asarray(weight, dtype=np.float32)
    lora_A = np.asarray(lora_A, dtype=np.float32)
    lora_B_q = np.asarray(lora_B_q, dtype=np.float32)
    lora_B_k = np.asarray(lora_B_k, dtype=np.float32)
    lora_B_v = np.asarray(lora_B_v, dtype=np.float32)
    lora_scaling = np.asarray(lora_scaling, dtype=np.float32)
    slot = np.asarray(token_to_slot).astype(np.int64)

    xT = np.ascontiguousarray(x.astype(NPBF16).T)       # (2048, 8192) bf16
    # wJ[j, p, k*512+o] = W^T[k*128+p, j*512+o]: 16 KB contiguous per
    # (j, partition) so the per-j DMA uses full-size descriptors.
    wT = weight.astype(NPBF16).T                        # (2048, 3072) bf16
    wJ = np.ascontiguousarray(
        wT.reshape(KC, P, OJ, 512).transpose(2, 1, 0, 3).reshape(OJ, P, KC * 512))
    # aT col = g*128 + l*16 + r; aQ pairs k-chunks for 1.5 KB lines
    aT = lora_A.transpose(1, 0, 2, 3).reshape(3 * GR, HIDDEN).T.astype(NPBF16)
    aQ = np.ascontiguousarray(
        aT.reshape(KC // 2, 2, P, 3 * GR).transpose(0, 2, 1, 3)
        .reshape(KC // 2, P, 2 * 3 * GR))
    # b row = l*16 + r, scaling folded in; columns = q | k | v
    bq = (lora_scaling[:, None, None] * lora_B_q).transpose(0, 2, 1).reshape(GR, Q_SIZE)
    bk = (lora_scaling[:, None, None] * lora_B_k).transpose(0, 2, 1).reshape(GR, KV_SIZE)
    bv = (lora_scaling[:, None, None] * lora_B_v).transpose(0, 2, 1).reshape(GR, KV_SIZE)
    bqkv = np.ascontiguousarray(
        np.concatenate([bq, bk, bv], axis=1).astype(NPBF16))  # (128, 3072)
    # routing mask [l*16+r, t]: 1 where slot[t] == l (scale already in B)
    onehot = (slot[None, :] == np.arange(MAX_LORAS)[:, None])          # (8, T)
    mask = np.repeat(onehot, RANK, axis=0).astype(NPBF16)              # (128, T)

    in_maps = []
    for c in range(N_CORES):
        sl = slice(c * T_CORE, (c + 1) * T_CORE)
        xc = xT[:, sl]  # (2048, 1024)
        xQ = np.ascontiguousarray(
            xc.reshape(KC // 2, 2, P, T_CORE).transpose(0, 2, 1, 3)
            .reshape(KC // 2, P, 2 * T_CORE))
        in_maps.append({
            "xQ": xQ,
            "wJ": wJ,
            "aQ": aQ,
            "bqkv": bqkv,
            "mask": np.ascontiguousarray(mask[:, sl]),
        })
    return in_maps


def kernel(**inputs):
    from concourse.bass_utils import run_bass_kernel_spmd
    if "nc" not in _NC_CACHE:
        _NC_CACHE["nc"] = build_nc()
    nc = _NC_CACHE["nc"]
    in_maps = prep_in_maps(**inputs)
    res = run_bass_kernel_spmd(nc, in_maps, core_ids=list(range(N_CORES)))
    return np.concatenate(
        [r["yb"].astype(np.float32) for r in res.results], axis=0)


# revision 30
# speedup vs baseline: 1.0220x; 1.0022x over previous
"""LoRA QKV parallel linear with per-token slot routing, on 8 TRN2 NeuronCores.

Data-parallel over tokens (8192 -> 1024 per core), weights replicated.
All matmul operands are bf16 (tolerance 2e-2 >> bf16 matmul error ~3.5e-3),
which halves HBM traffic / SBUF footprint vs f32 and enables the fast
weight-load path; PSUM accumulation stays fp32. The kernel is PE-bound:
912 matmuls x 512 cols ~= 197us/core at full rate, and the schedule keeps
the PE array >90% busy.

Per core:
  warm-up: ~16 junk matmuls on a zeroed tile while the first input DMAs
           land, so the HAM clock-gate is at full rate for the real stream.
  phase 1: hT[(g,l,r), t] = A-chunk-stationary matmuls over k, directly in
           the [rank, token] orientation (no PE transposes); each bank is
           masked with a host-precomputed one-hot routing mask
           m[(l,r), t] = (slot[t] == l) and downcast to bf16 on DVE as soon
           as its accumulation closes (lora_scaling is folded into B).
  phase 2: j-outer over 512-wide o-chunks so w chunk j is only needed when
           phase 2 reaches it; per 128-token tile i, 16 k-chunk matmuls
           accumulate into one PSUM bank (8-bank rotation, offset so the
           first blocks don't wait on the mask ops), closed by the LoRA
           delta matmul (hT-slice stationary @ B). Output is written bf16
           and upcast on host.

Inputs are host-swizzled so every DMA moves multi-KB contiguous
per-partition lines (descriptor efficiency sets the effective input
bandwidth), and the issue order matches the consumption order.
"""

import numpy as np
import ml_dtypes

import concourse.bacc as bacc
import concourse.mybir as mybir
import concourse.tile as tile

HIDDEN = 2048
Q_SIZE = 2048
KV_SIZE = 512
OUT = Q_SIZE + 2 * KV_SIZE  # 3072
MAX_LORAS = 8
RANK = 16
T = 8192
N_CORES = 8
T_CORE = T // N_CORES  # 1024

P = 128
KC = HIDDEN // P          # 16 k-chunks
OJ = OUT // 512           # 6 output chunks of 512
NT = T_CORE // P          # 8 token tiles per core
GR = MAX_LORAS * RANK     # 128 = all (slot, rank) pairs for one target group
F32 = mybir.dt.float32
BF16 = mybir.dt.bfloat16
NPBF16 = ml_dtypes.bfloat16

_NC_CACHE = {}


def build_nc():
    """Build the SPMD Bass program (same program on every core)."""
    nc = bacc.Bacc("TRN2", target_bir_lowering=False, debug=False, num_devices=N_CORES)

    # x/A/w are pre-swizzled host-side so every DMA moves multi-KB contiguous
    # per-partition lines (descriptor efficiency sets input bandwidth):
    #   xQ[m, p, c*1024+t] = x^T[(2m+c)*128+p, t]            (4 KB lines)
    #   aQ[m, p, c*384+gr] = A^T[(2m+c)*128+p, gr]           (1.5 KB lines)
    #   wJ[j, p, k*512+o]  = W^T[k*128+p, j*512+o]           (8 KB lines)
    xQ = nc.dram_tensor("xQ", [KC // 2, P, 2 * T_CORE], BF16,
                        kind="ExternalInput").ap()
    aQ = nc.dram_tensor("aQ", [KC // 2, P, 2 * 3 * GR], BF16,
                        kind="ExternalInput").ap()
    wJ = nc.dram_tensor("wJ", [OJ, P, KC * 512], BF16, kind="ExternalInput").ap()
    bqkv = nc.dram_tensor("bqkv", [GR, OUT], BF16, kind="ExternalInput").ap()
    mask = nc.dram_tensor("mask", [GR, T_CORE], BF16, kind="ExternalInput").ap()
    yb = nc.dram_tensor("yb", [T_CORE, OUT], BF16, kind="ExternalOutput").ap()

    with tile.TileContext(nc) as tc:
        with (
            tc.tile_pool(name="xsb", bufs=1) as xpool,
            tc.tile_pool(name="wsb", bufs=1) as wpool,
            tc.tile_pool(name="asb", bufs=1) as apool,
            tc.tile_pool(name="bsb", bufs=1) as bpool,
            tc.tile_pool(name="msb", bufs=1) as mpool,
            tc.tile_pool(name="ht", bufs=1) as htpool,
            tc.tile_pool(name="o", bufs=12) as opool,
            tc.tile_pool(name="ps", bufs=1, space="PSUM") as pspool,
        ):
            # --- resident inputs (DMA'd in chunks; tile deps order the MMs) ---
            # Issue order matters: phase 1 streams x/A chunks immediately;
            # mask+B are needed at the first delta (~+35us); w chunk j is
            # needed only when phase 2 reaches o-chunk j (j-outer loop).
            xsb = xpool.tile([P, KC * T_CORE], BF16)    # free idx = k*1024 + t
            asb = apool.tile([P, KC * 3 * GR], BF16)    # free idx = k*384 + g*128 + lr
            msb = mpool.tile([P, T_CORE], BF16)         # routing mask [lr, t]
            bsb = bpool.tile([P, OUT], BF16)            # [lr, o] = bq|bk|bv
            # w stored per o-chunk: free idx = j*(16*512) + k*512 + o.
            # 8 k-chunks per DMA: 8 KB contiguous per partition line.
            wsb = wpool.tile([P, KC * OUT], BF16)

            def dma_w(j, kq):   # kq in 0..1, 8 k-chunks each
                nc.sync.dma_start(
                    wsb[:, (j * KC + kq * 8) * 512:(j * KC + (kq + 1) * 8) * 512],
                    wJ[j, :, kq * 8 * 512:(kq + 1) * 8 * 512])

            def dma_xa(m):      # k-chunk pair m covers k = 2m, 2m+1
                nc.sync.dma_start(
                    xsb[:, 2 * m * T_CORE:2 * (m + 1) * T_CORE], xQ[m, :, :])
                nc.sync.dma_start(
                    asb[:, 2 * m * 3 * GR:2 * (m + 1) * 3 * GR], aQ[m, :, :])

            # Issue order = arrival order. Phase 1 consumes x/A chunk k at
            # ~1.3us per chunk; w_j0 must be complete when phase 2 starts
            # (right at phase-1 end), so its sub-DMAs go mid-stream. The
            # first pair is split in half so phase 1 starts sooner.
            nc.sync.dma_start(xsb[:, 0:T_CORE], xQ[0, :, 0:T_CORE])
            nc.sync.dma_start(asb[:, 0:3 * GR], aQ[0, :, 0:3 * GR])
            nc.sync.dma_start(xsb[:, T_CORE:2 * T_CORE], xQ[0, :, T_CORE:])
            nc.sync.dma_start(asb[:, 3 * GR:2 * 3 * GR], aQ[0, :, 3 * GR:])
            for m in range(1, 4):
                dma_xa(m)
            nc.sync.dma_start(msb[:], mask[:, :])
            nc.sync.dma_start(bsb[:], bqkv[:, :])
            dma_w(0, 0)
            for m in range(4, 6):
                dma_xa(m)
            dma_w(0, 1)
            for m in range(6, 8):
                dma_xa(m)
            for j in range(1, OJ):
                for kq in range(2):
                    dma_w(j, kq)

            # --- PE warm-up: ~16 junk matmuls on a zeroed tile while the
            # first input DMAs land, so the HAM clock-gate is at full rate
            # when the real stream starts ---
            warm = mpool.tile([P, 512], BF16, name="warm_src")
            nc.vector.memset(warm[:], 0.0)
            wps = pspool.tile([P, 512], F32, name="warm_ps", tag="ps6")
            for _ in range(16):
                nc.tensor.matmul(wps[:], lhsT=warm[:, 0:P], rhs=warm[:],
                                 start=True, stop=True)

            # --- phase 1: hT[g][lr, t], A-chunk stationary, x moving;
            # each (g,th) bank is masked (one-hot) + downcast to bf16 on DVE
            # as soon as its accumulation closes ---
            hps = [pspool.tile([P, 512], F32, name=f"hps{n}", tag=f"ps{n}") for n in range(6)]
            hT = htpool.tile([P, 3 * T_CORE], BF16)     # free idx = g*1024 + t
            for k in range(KC):
                for g in range(3):
                    for th in range(2):
                        nc.tensor.matmul(
                            hps[g * 2 + th][:],
                            lhsT=asb[:, k * 3 * GR + g * GR:
                                     k * 3 * GR + (g + 1) * GR],
                            rhs=xsb[:, k * T_CORE + th * 512:
                                    k * T_CORE + (th + 1) * 512],
                            start=(k == 0), stop=(k == KC - 1))
                        if k == KC - 1:
                            nc.vector.tensor_tensor(
                                hT[:, g * T_CORE + th * 512:
                                   g * T_CORE + (th + 1) * 512],
                                hps[g * 2 + th][:],
                                msb[:, th * 512:(th + 1) * 512],
                                op=mybir.AluOpType.mult)

            # --- phase 2: y tile [t128, o512]; j-outer so w chunk j is only
            # needed once phase 2 reaches it (relaxes the DMA deadline) ---
            # delta source group per o-chunk j: q,q,q,q,k,v
            jg = [0, 0, 0, 0, 1, 2]
            for j in range(OJ):
                for i in range(NT):
                    n = j * NT + i
                    ops = pspool.tile([P, 512], F32, name=f"ops{n}",
                                      tag=f"ps{(n + 6) % 8}")
                    for k in range(KC):
                        nc.tensor.matmul(
                            ops[:],
                            lhsT=xsb[:, k * T_CORE + i * P:
                                     k * T_CORE + (i + 1) * P],
                            rhs=wsb[:, (j * KC + k) * 512:
                                    (j * KC + k + 1) * 512],
                            start=(k == 0), stop=False)
                    nc.tensor.matmul(
                        ops[:],
                        lhsT=hT[:, jg[j] * T_CORE + i * P:
                                jg[j] * T_CORE + (i + 1) * P],
                        rhs=bsb[:, j * 512:(j + 1) * 512],
                        start=False, stop=True)
                    osb = opool.tile([P, 512], BF16, name=f"osb{i}_{j}", tag="o")
                    nc.vector.tensor_copy(osb[:], ops[:])
                    nc.sync.dma_start(
                        yb[i * P:(i + 1) * P, j * 512:(j + 1) * 512], osb[:])
    nc.compile()
    return nc


def prep_in_maps(x, weight, lora_A, lora_B_q, lora_B_k, lora_B_v,
                 lora_scaling, token_to_slot):
    x = np.asarray(x, dtype=np.float32)
    weight = np.asarray(weight, dtype=np.float32)
    lora_A = np.asarray(lora_A, dtype=np.float32)
    lora_B_q = np.asarray(lora_B_q, dtype=np.float32)
    lora_B_k = np.asarray(lora_B_k, dtype=np.float32)
    lora_B_v = np.asarray(lora_B_v, dtype=np.float32)
    lora_scaling = np.asarray(lora_scaling, dtype=np.float32)
    slot = np.asarray(token_to_slot).astype(np.int64)

    xT = np.ascontiguousarray(x.astype(NPBF16).T)       # (2048, 8192) bf16
    # wJ[j, p, k*512+o] = W^T[k*128+p, j*512+o]: 16 KB contiguous per
    # (j, partition) so the per-j DMA uses full-size descriptors.
    wT = weight.astype(NPBF16).T                        # (2048, 3072) bf16
    wJ = np.ascontiguousarray(
        wT.reshape(KC, P, OJ, 512).transpose(2, 1, 0, 3).reshape(OJ, P, KC * 512))
    # aT col = g*128 + l*16 + r; aQ pairs k-chunks for 1.5 KB lines
    aT = lora_A.transpose(1, 0, 2, 3).reshape(3 * GR, HIDDEN).T.astype(NPBF16)
    aQ = np.ascontiguousarray(
        aT.reshape(KC // 2, 2, P, 3 * GR).transpose(0, 2, 1, 3)
        .reshape(KC // 2, P, 2 * 3 * GR))
    # b row = l*16 + r, scaling folded in; columns = q | k | v
    bq = (lora_scaling[:, None, None] * lora_B_q).transpose(0, 2, 1).reshape(GR, Q_SIZE)
    bk = (lora_scaling[:, None, None] * lora_B_k).transpose(0, 2, 1).reshape(GR, KV_SIZE)
    bv = (lora_scaling[:, None, None] * lora_B_v).transpose(0, 2, 1).reshape(GR, KV_SIZE)
    bqkv = np.ascontiguousarray(
        np.concatenate([bq, bk, bv], axis=1).astype(NPBF16))  # (128, 3072)
    # routing mask [l*16+r, t]: 1 where slot[t] == l (scale already in B)
    onehot = (slot[None, :] == np.arange(MAX_LORAS)[:, None])          # (8, T)
    mask = np.repeat(onehot, RANK, axis=0).astype(NPBF16)              # (128, T)

    in_maps = []
    for c in range(N_CORES):
        sl = slice(c * T_CORE, (c + 1) * T_CORE)
        xc = xT[:, sl]  # (2048, 1024)
        xQ = np.ascontiguousarray(
            xc.reshape(KC // 2, 2, P, T_CORE).transpose(0, 2, 1, 3)
            .reshape(KC // 2, P, 2 * T_CORE))
        in_maps.append({
            "xQ": xQ,
            "wJ": wJ,
            "aQ": aQ,
            "bqkv": bqkv,
            "mask": np.ascontiguousarray(mask[:, sl]),
        })
    return in_maps


def kernel(**inputs):
    from concourse.bass_utils import run_bass_kernel_spmd
    if "nc" not in _NC_CACHE:
        _NC_CACHE["nc"] = build_nc()
    nc = _NC_CACHE["nc"]
    in_maps = prep_in_maps(**inputs)
    res = run_bass_kernel_spmd(nc, in_maps, core_ids=list(range(N_CORES)))
    return np.concatenate(
        [r["yb"].astype(np.float32) for r in res.results], axis=0)
